# revision 58
# baseline (speedup 1.0000x reference)
# ContextQueryAttention (BiDAF-style) Trainium2 Bass/Tile kernel.
#
# Full-input contract: kernel(**inputs) takes the full arrays
#   context [32, 2048, 128] f32, query [32, 128, 128] f32,
#   w [384] f32, query_mask [32, 128] i32
# and returns out [32, 2048, 512] f32.
#
# Sharding: batch B=32 split 4-per-core across 8 NeuronCores (pure data
# parallel, no collectives).
#
# Math (per batch, C=2048, Q=128, D=128):
#   S[c,q] = ctx[c]@w1 + query[q]@w2 + (ctx[c]*w3)@query[q]
#          = alpha[c] + beta[q] + G[c,q]
#   a = softmax_q(S + maskadd);  c2q = a @ query
#   m[c] = max_q(S + maskadd);   b = softmax_c(m); q2c = b @ ctx
#   out = [ctx | c2q | ctx*c2q | ctx*q2c]
#
# Design notes (cost-model driven; DMA floor = 20.2 MiB/core at
# 360 GB/s = 59.0 us, the DMA engine runs gap-free start to end and the
# total is floor + fixed startup/drain only):
#  * alpha[c] cancels in softmax_q -> row softmax runs on T = G + beta'
#    (beta' = beta + mask_add) fused into the ACT exp bias in [q, c] layout.
#  * |S| = O(5), so exp() without max-subtraction is exact to fp32 roundoff.
#  * bf16 everywhere precision allows (rel-err budget 2e-2, this costs
#    ~5e-4): ctxT/qw3T for the G matmul, E^T = exp(T^T), the c2q matmuls
#    and E transposes -> 1 cyc/row on the PE instead of 4 (fp32).
#  * max_q E per c-tile via PE-transpose of E^T; 4 tiles transposed into
#    one PSUM bank and reduced with a single 3D reduce_max.
#  * u = sum_c e_m[c]*ctx[c] computed transposed: stationary ctx tile,
#    moving e_m column -> N=1 matmuls (~free on PE).
#  * out[:, :, 0:128] == ctx exactly, so it ships as dependency-free
#    DRAM->DRAM copies on the gpsimd SWDGE queue; they fill every DMA
#    idle window (the cost model charges only output bytes).
#  * Group-pipelined batch body: per group of 4 c-tiles, G -> exp -> cq/
#    etr -> scales/out3 -> store, with the next group's ctx transposes
#    staged one iteration ahead.  First store of a batch comes ~2.5 us
#    after the batch starts, so the DMA queue never starves at batch
#    boundaries (DMA engine ends up >94% busy, gap-free after startup).
#  * One SBUF assembly tile per batch [128, 16*512]; ctx loads land in its
#    first column block; stores are per-group (cols 128:384) plus 4
#    out4 pieces (cols 384:512).  All loads are issued before any store
#    on the SP queue so a waiting store never blocks a later load; w and
#    the out1 d2d copies ride the SWDGE queue (no HWDGE contention).
#  * Elementwise work is spread: exp/scales on ACT, muls/reduces/recips
#    on DVE, part of out3/out4 on gpsimd (Pool).  Paired reciprocals
#    (2 Z columns per DVE op) halve the per-tile recip overhead.
#
# PSUM (8 banks): big 2 (head / ctx transposes / G) + etr 2 (E-transpose
# groups) + cq 3 (c2q results 2-per-bank + alpha columns) + tail 1
# (u accumulation chain, zb, q2c row, broadcast).

import numpy as np

C = 2048
Q = 128
D = 128
B_TOTAL = 32
N_CORES = 8
B_LOCAL = B_TOTAL // N_CORES  # 4
N_CT = C // 128  # 16 c-tiles per batch
N_G = 4  # groups of 4 c-tiles

_compiled = None


def _build():
    import concourse.bacc as bacc
    import concourse.tile as tile
    import concourse.mybir as mybir
    from concourse import masks

    f32 = mybir.dt.float32
    i32 = mybir.dt.int32

    nc = bacc.Bacc(
        "TRN2",
        target_bir_lowering=False,
        debug=False,
        num_devices=N_CORES,
    )

    ctx_d = nc.dram_tensor("context", [B_LOCAL, C, D], f32, kind="ExternalInput").ap()
    qry_d = nc.dram_tensor("query", [B_LOCAL, Q, D], f32, kind="ExternalInput").ap()
    w_d = nc.dram_tensor("w", [3 * D], f32, kind="ExternalInput").ap()
    msk_d = nc.dram_tensor("query_mask", [B_LOCAL, Q], i32, kind="ExternalInput").ap()
    out_d = nc.dram_tensor("out", [B_LOCAL, C, 4 * D], f32, kind="ExternalOutput").ap()

    with tile.TileContext(nc) as tc:
        _kernel_body(tc, out_d, ctx_d, qry_d, w_d, msk_d, mybir, masks)

    nc.compile()
    return nc


def _kernel_body(tc, out_d, ctx_d, qry_d, w_d, msk_d, mybir, masks):
    from contextlib import ExitStack

    nc = tc.nc
    f32 = mybir.dt.float32
    bf16 = mybir.dt.bfloat16
    f32r = mybir.dt.float32r
    i32 = mybir.dt.int32
    AFT = mybir.ActivationFunctionType
    Alu = mybir.AluOpType
    AX = mybir.AxisListType.X

    es = ExitStack()
    with es:
        # ---- pools ----
        consts = es.enter_context(tc.tile_pool(name="consts", bufs=1))
        outp = es.enter_context(tc.tile_pool(name="outp", bufs=4))
        bigs = es.enter_context(tc.tile_pool(name="bigs", bufs=2))
        meds = es.enter_context(tc.tile_pool(name="meds", bufs=2))
        cols = es.enter_context(tc.tile_pool(name="cols", bufs=8))
        ps_big = es.enter_context(tc.tile_pool(name="ps_big", bufs=2, space="PSUM"))
        ps_etr = es.enter_context(tc.tile_pool(name="ps_etr", bufs=2, space="PSUM"))
        ps_cq = es.enter_context(tc.tile_pool(name="ps_cq", bufs=3, space="PSUM"))
        ps_tail = es.enter_context(tc.tile_pool(name="ps_tail", bufs=1, space="PSUM"))

        # ---- constants ----
        # One SWDGE DMA for all of w as a contiguous row (1 descriptor; the
        # [d,1] column layout pays 7ns/desc min-transfer x 384), issued
        # before the identity builders so w is ready early.  Columns are
        # recovered with PE transposes below.
        w_row = consts.tile([1, 3 * D], f32)
        nc.gpsimd.dma_start(out=w_row[:], in_=w_d.rearrange("n -> () n"))
        ident = consts.tile([128, 128], f32)
        ident_bf = consts.tile([128, 128], bf16)
        ones_col = consts.tile([128, 1], f32)
        nc.vector.memset(ones_col[:], 1.0)
        ones_row_bf = consts.tile([1, 128], bf16)
        nc.vector.memset(ones_row_bf[:], 1.0)
        # all 4 batches' masks in one contiguous row: 1 descriptor instead of
        # 4x128 (the [q,1] column layout pays 7ns/desc min-transfer x 128)
        mask_rows = consts.tile([1, 4 * Q], i32)
        # p-major c mapping: c-tile i holds context rows c = p*16 + i, so
        # each partition covers 16 consecutive rows = 8 KiB contiguous DRAM.
        # That lets the bf16 casting ctx load use 4 KiB descriptors (full
        # DMA rate, halving ctx load bytes); compute is bf16-safe since the
        # exact f32 out1 block ships via DRAM->DRAM copy.
        ctx_v = ctx_d.rearrange("b (p i) d -> b p i d", i=N_CT)
        ctx_flat = ctx_d.rearrange("b (p i) d -> b p (i d)", i=N_CT)
        out_v = out_d.rearrange("b (p i) f -> b p i f", i=N_CT)

        # ---------- loads ----------
        # SP carries only the tiny qry/mask loads (stores dominate it later);
        # the bf16 ctx casting loads and the out1 d2d copies ride the SWDGE
        # queue, dependency-free, filling every DMA idle window.
        gts = []
        gvs = []
        qfs = []
        cbs = []
        for b in range(B_LOCAL):
            qf = meds.tile([128, 128], f32, tag="qf", bufs=4)
            gt = outp.tile([128, N_CT * 384], f32, tag="out")
            gv = gt.rearrange("p (i f) -> p i f", i=N_CT)
            cb = meds.tile([128, N_CT * 128], bf16, tag="ctxbf", bufs=4)
            gts.append(gt)
            gvs.append(gv)
            qfs.append(qf)
            cbs.append(cb)

        nc.sync.dma_start(out=qfs[0][:], in_=qry_d[0])
        # ctx_bf0 generates right after w_row on the SWDGE queue; d2d0 rides
        # the HWDGE queue (eligible ~2.6us) to bridge the head until the
        # SWDGE pipeline spins up
        nc.gpsimd.dma_start(out=cbs[0][:], in_=ctx_flat[0])
        nc.sync.dma_start(out=out_v[0][:, :, 0:128], in_=ctx_v[0])
        masks.make_identity(nc, ident[:])
        masks.make_identity(nc, ident_bf[:])
        nc.sync.dma_start(
            out=mask_rows[:], in_=msk_d.rearrange("b q -> () (b q)")
        )
        for b in range(1, B_LOCAL):
            nc.sync.dma_start(out=qfs[b][:], in_=qry_d[b])
        # recover w columns: PE transposes of the row slices + SBUF copies
        wps = ps_big.tile([128, 512], f32, tag="big")
        for k in range(3):
            nc.tensor.transpose(
                wps[:, k : k + 1], w_row[0:1, k * D : (k + 1) * D], ident[0:1, 0:1]
            )
        w1_col = consts.tile([128, 1], bf16)
        nc.vector.tensor_copy(w1_col[:], wps[:, 0:1])  # f32 -> bf16
        w2_col = consts.tile([128, 1], f32)
        nc.vector.tensor_copy(w2_col[:], wps[:, 1:2])
        w3_col = consts.tile([128, 1], f32)
        nc.vector.tensor_copy(w3_col[:], wps[:, 2:3])
        # SWDGE queue: remaining bf16 ctx casts + out1 d2d copies,
        # interleaved so the early ones are eligible as soon as possible
        nc.gpsimd.dma_start(out=cbs[1][:], in_=ctx_flat[1])
        nc.gpsimd.dma_start(out=out_v[1][:, :, 0:128], in_=ctx_v[1])
        nc.gpsimd.dma_start(out=cbs[2][:], in_=ctx_flat[2])
        nc.gpsimd.dma_start(out=out_v[2][:, :, 0:128], in_=ctx_v[2])
        nc.gpsimd.dma_start(out=cbs[3][:], in_=ctx_flat[3])
        nc.gpsimd.dma_start(out=out_v[3][:, :, 0:128], in_=ctx_v[3])

        for b in range(B_LOCAL):
            gv = gvs[b]
            cbv = cbs[b].rearrange("p (i d) -> p i d", i=N_CT)

            def ctx_blk(i):
                return cbv[:, i, :]

            def o_blk(i, k):
                return gv[:, i, (k - 1) * 128 : k * 128]

            # ---------- query prep ----------
            rhs_aug = meds.tile([128, 129], bf16, tag="rhs")
            nc.vector.tensor_copy(rhs_aug[:, 0:128], qfs[b][:])  # f32 -> bf16
            nc.vector.memset(rhs_aug[:, 128:129], 1.0)
            mrow_f = cols.tile([1, Q], f32, tag="mrowf")
            nc.vector.tensor_copy(
                mrow_f[:], mask_rows[0:1, b * Q : (b + 1) * Q]
            )  # int -> float cast

            # headA: qT (cols 0:128) + beta col (128) + mask col (129)
            headA = ps_big.tile([128, 512], f32, tag="big")
            nc.tensor.transpose(headA[:, 0:128], qfs[b][:], ident[:])
            nc.tensor.transpose(headA[:, 129:130], mrow_f[:], ident[0:1, 0:1])
            qT = meds.tile([128, 128], f32, tag="qT")
            nc.vector.tensor_copy(qT[:], headA[:, 0:128])
            qw3T = meds.tile([128, 128], bf16, tag="qw3T")
            nc.vector.tensor_scalar_mul(qw3T[:], qT[:], w3_col[:])
            madd_col = cols.tile([128, 1], f32, tag="madd")
            nc.vector.tensor_scalar(
                madd_col[:], headA[:, 129:130], 1.0, 1.0e9,
                op0=Alu.subtract, op1=Alu.mult,
            )
            nc.tensor.matmul(
                headA[:, 128:129], qT[:], w2_col[:], start=True, stop=True
            )
            beta_col = cols.tile([128, 1], f32, tag="beta")
            nc.vector.tensor_add(beta_col[:], madd_col[:], headA[:, 128:129])

            # per-batch tiles
            ctxT = bigs.tile([128, C], bf16, tag="ctxT")
            e_t = bigs.tile([128, C], bf16, tag="et")
            e_alpha = meds.tile([128, N_CT], f32, tag="ealpha")
            # bf16: pairs with the bf16 stationary ctx tile in the u matmuls
            e_m = meds.tile([128, N_CT], bf16, tag="em")
            # tail bank: u chain col 0, zb col 1, q2c row 2:130, bc 130:258
            tail_ps = ps_tail.tile([128, 512], f32, tag="tail")

            def stage_transposes(g):
                tr_ps = ps_big.tile([128, 512], bf16, tag="big")
                for j in range(4):
                    nc.tensor.transpose(
                        tr_ps[:, j * 128 : (j + 1) * 128],
                        ctx_blk(g * 4 + j),
                        ident_bf[:],
                    )
                # bf16 PSUM->SBUF copy hits DVE's 2x_1p fast path (392ns);
                # keeping all 4 on DVE takes them off the ACT group cadence
                nc.vector.tensor_copy(ctxT[:, g * 512 : (g + 1) * 512], tr_ps[:])

            # ---------- group-pipelined main loop ----------
            # Per group: G -> exp -> cq/etr -> scales/out3 -> store, with the
            # next group's ctx transposes staged one iteration ahead so the
            # PSUM->SBUF copy and exp latency never stall the PE.
            stage_transposes(0)
            for g in range(N_G):
                # G^T for this group (ctxT[g] copied last iteration)
                st_ps = ps_big.tile([128, 512], f32, tag="big")
                nc.tensor.matmul(
                    st_ps[:],
                    qw3T[:],
                    ctxT[:, g * 512 : (g + 1) * 512],
                    start=True,
                    stop=True,
                )
                if g + 1 < N_G:
                    stage_transposes(g + 1)
                nc.scalar.activation(
                    out=e_t[:, g * 512 : (g + 1) * 512],
                    in_=st_ps[:],
                    func=AFT.Exp,
                    bias=beta_col[:],
                    scale=1.0,
                )
                # cq PSUM allocs; alpha columns ride in cq2a cols 258:262
                cq2a = ps_cq.tile([128, 262], f32, tag="cq")
                for jj in range(4):
                    i = 4 * g + jj
                    nc.tensor.matmul(
                        cq2a[:, 258 + jj : 259 + jj],
                        ctxT[:, i * 128 : (i + 1) * 128],
                        w1_col[:],
                        start=True,
                        stop=True,
                    )
                # u chain for the previous group (e_m ready by then)
                if g > 0:
                    for i in range(4 * (g - 1), 4 * g):
                        nc.tensor.matmul(
                            tail_ps[:, 0:1],
                            ctx_blk(i),
                            e_m[:, i : i + 1],
                            start=(i == 0),
                            stop=False,
                        )
                etr = ps_etr.tile([128, 512], bf16, tag="etr")
                rzs = {}
                cq_slots = {}
                cq2 = cq2a
                for j in range(4):
                    i = 4 * g + j
                    if j == 2:
                        cq2 = ps_cq.tile([128, 262], f32, tag="cq")
                    cqs = cq2[:, 129 * (j % 2) : 129 * (j % 2) + 129]
                    cq_slots[j] = cqs
                    et_sl = e_t[:, i * 128 : (i + 1) * 128]
                    nc.tensor.matmul(cqs, et_sl, rhs_aug[:], start=True, stop=True)
                    nc.tensor.transpose(
                        etr[:, j * 128 : (j + 1) * 128], et_sl, ident_bf[:]
                    )
                    if j % 2 == 1:
                        # one reciprocal per pair: Z cols sit at 128 and 257
                        rz2 = cols.tile([128, 2], f32, tag="rz")
                        zv = cq2[:, 0:258].rearrange("p (k n) -> p k n", k=2)[
                            :, :, 128
                        ]
                        nc.vector.reciprocal(rz2[:], zv)
                        rzs[j - 1] = rz2[:, 0:1]
                        rzs[j] = rz2[:, 1:2]
                for j in range(4):
                    i = 4 * g + j
                    # c2q = (E @ [qry|1]) / Z  (ACT copy with per-partition scale)
                    nc.scalar.activation(
                        out=o_blk(i, 1),
                        in_=cq_slots[j][:, 0:128],
                        func=AFT.Copy,
                        scale=rzs[j],
                    )
                # e^alpha for this group's 4 tiles (not on the store path)
                nc.scalar.activation(
                    out=e_alpha[:, 4 * g : 4 * g + 4],
                    in_=cq2a[:, 258:262],
                    func=AFT.Exp,
                )
                # row max over q of the 4 transposed tiles, then e_m
                maxE = cols.tile([128, 4], f32, tag="maxE")
                nc.vector.reduce_max(
                    out=maxE[:], in_=etr.rearrange("p (j q) -> p j q", j=4), axis=AX
                )
                nc.vector.tensor_mul(
                    e_m[:, 4 * g : 4 * g + 4], e_alpha[:, 4 * g : 4 * g + 4], maxE[:]
                )
                # out3 = ctx * c2q (SBUF-only: gpsimd can help)
                for j in range(4):
                    i = 4 * g + j
                    eng = nc.gpsimd if j >= 2 else nc.vector
                    eng.tensor_mul(o_blk(i, 2), ctx_blk(i), o_blk(i, 1))
                # this group's [c2q | ctx*c2q] is final: ship it
                # (batch 0 group 0 in two halves so DMA starts sooner)
                if b == 0 and g == 0:
                    nc.sync.dma_start(
                        out=out_v[b][:, 0:2, 128:384], in_=gv[:, 0:2, 0:256]
                    )
                    nc.sync.dma_start(
                        out=out_v[b][:, 2:4, 128:384], in_=gv[:, 2:4, 0:256]
                    )
                else:
                    nc.sync.dma_start(
                        out=out_v[b][:, 4 * g : 4 * g + 4, 128:384],
                        in_=gv[:, 4 * g : 4 * g + 4, 0:256],
                    )
            for i in range(4 * (N_G - 1), N_CT):
                nc.tensor.matmul(
                    tail_ps[:, 0:1],
                    ctx_blk(i),
                    e_m[:, i : i + 1],
                    start=False,
                    stop=(i == N_CT - 1),
                )

            # ---------- q2c epilogue ----------
            zsum = cols.tile([128, 1], f32, tag="zsum")
            nc.vector.reduce_sum(out=zsum[:], in_=e_m[:], axis=AX)
            nc.tensor.matmul(
                tail_ps[0:1, 1:2], zsum[:], ones_col[:], start=True, stop=True
            )
            u_sb = cols.tile([128, 1], f32, tag="usb")
            nc.vector.tensor_copy(u_sb[:], tail_ps[:, 0:1])
            nc.tensor.transpose(tail_ps[0:1, 2:130], u_sb[:], ident[:])
            rzb = cols.tile([1, 1], f32, tag="rzb")
            nc.vector.reciprocal(rzb[:], tail_ps[0:1, 1:2])
            q2c_row = cols.tile([1, 128], bf16, tag="q2crow")
            nc.scalar.activation(
                out=q2c_row[:], in_=tail_ps[0:1, 2:130], func=AFT.Copy, scale=rzb[:]
            )
            nc.tensor.matmul(
                tail_ps[:, 130:258], ones_row_bf[:], q2c_row[:], start=True, stop=True
            )
            q2c_sb = meds.tile([128, 128], f32, tag="q2csb")
            nc.vector.tensor_copy(q2c_sb[:], tail_ps[:, 130:258])

            # ---------- out4 + remaining stores (4 pieces) ----------
            # 3 DVE + 1 gpsimd mul per piece; each piece ships on completion
            for piece in range(4):
                for j in range(4):
                    i = 4 * piece + j
                    eng = nc.gpsimd if j == 1 else nc.vector
                    eng.tensor_mul(o_blk(i, 3), ctx_blk(i), q2c_sb[:])
                nc.sync.dma_start(
                    out=out_v[b][:, 4 * piece : 4 * piece + 4, 384:512],
                    in_=gv[:, 4 * piece : 4 * piece + 4, 256:384],
                )


def kernel(**inputs):
    global _compiled
    from concourse.bass_utils import run_bass_kernel_spmd

    context = np.ascontiguousarray(inputs["context"], dtype=np.float32)
    query = np.ascontiguousarray(inputs["query"], dtype=np.float32)
    w = np.ascontiguousarray(inputs["w"], dtype=np.float32)
    qmask = np.ascontiguousarray(inputs["query_mask"], dtype=np.int32)

    if _compiled is None:
        _compiled = _build()
    nc = _compiled

    core_ids = list(range(N_CORES))
    in_maps = []
    for k in core_ids:
        sl = slice(k * B_LOCAL, (k + 1) * B_LOCAL)
        in_maps.append(
            {
                "context": context[sl],
                "query": query[sl],
                "w": w,
                "query_mask": qmask[sl],
            }
        )

    res = run_bass_kernel_spmd(nc, in_maps, core_ids)
    outs = [res.results[k]["out"] for k in range(N_CORES)]
    return np.concatenate(outs, axis=0)


# revision 59
# speedup vs baseline: 1.0475x; 1.0475x over previous
# ContextQueryAttention (BiDAF-style) Trainium2 Bass/Tile kernel.
#
# Full-input contract: kernel(**inputs) takes the full arrays
#   context [32, 2048, 128] f32, query [32, 128, 128] f32,
#   w [384] f32, query_mask [32, 128] i32
# and returns out [32, 2048, 512] f32.
#
# Sharding: batch B=32 split 4-per-core across 8 NeuronCores (pure data
# parallel, no collectives).
#
# Math (per batch, C=2048, Q=128, D=128):
#   S[c,q] = ctx[c]@w1 + query[q]@w2 + (ctx[c]*w3)@query[q]
#          = alpha[c] + beta[q] + G[c,q]
#   a = softmax_q(S + maskadd);  c2q = a @ query
#   m[c] = max_q(S + maskadd);   b = softmax_c(m); q2c = b @ ctx
#   out = [ctx | c2q | ctx*c2q | ctx*q2c]
#
# Design notes (cost-model driven; DMA floor = 20.2 MiB/core at
# 360 GB/s = 59.0 us, the DMA engine runs gap-free start to end and the
# total is floor + fixed startup/drain only):
#  * alpha[c] cancels in softmax_q -> row softmax runs on T = G + beta'
#    (beta' = beta + mask_add) fused into the ACT exp bias in [q, c] layout.
#  * |S| = O(5), so exp() without max-subtraction is exact to fp32 roundoff.
#  * bf16 everywhere precision allows (rel-err budget 2e-2, this costs
#    ~5e-4): ctxT/qw3T for the G matmul, E^T = exp(T^T), the c2q matmuls
#    and E transposes -> 1 cyc/row on the PE instead of 4 (fp32).
#  * max_q E per c-tile via PE-transpose of E^T; 4 tiles transposed into
#    one PSUM bank and reduced with a single 3D reduce_max.
#  * u = sum_c e_m[c]*ctx[c] computed transposed: stationary ctx tile,
#    moving e_m column -> N=1 matmuls (~free on PE).
#  * out[:, :, 0:128] == ctx exactly, so it ships as dependency-free
#    DRAM->DRAM copies on the gpsimd SWDGE queue; they fill every DMA
#    idle window (the cost model charges only output bytes).
#  * Group-pipelined batch body: per group of 4 c-tiles, G -> exp -> cq/
#    etr -> scales/out3 -> store, with the next group's ctx transposes
#    staged one iteration ahead.  First store of a batch comes ~2.5 us
#    after the batch starts, so the DMA queue never starves at batch
#    boundaries (DMA engine ends up >94% busy, gap-free after startup).
#  * One SBUF assembly tile per batch [128, 16*512]; ctx loads land in its
#    first column block; stores are per-group (cols 128:384) plus 4
#    out4 pieces (cols 384:512).  All loads are issued before any store
#    on the SP queue so a waiting store never blocks a later load; w and
#    the out1 d2d copies ride the SWDGE queue (no HWDGE contention).
#  * Elementwise work is spread: exp/scales on ACT, muls/reduces/recips
#    on DVE, part of out3/out4 on gpsimd (Pool).  Paired reciprocals
#    (2 Z columns per DVE op) halve the per-tile recip overhead.
#
# PSUM (8 banks): big 2 (head / ctx transposes / G) + etr 2 (E-transpose
# groups) + cq 3 (c2q results 2-per-bank + alpha columns) + tail 1
# (u accumulation chain, zb, q2c row, broadcast).

import numpy as np

C = 2048
Q = 128
D = 128
B_TOTAL = 32
N_CORES = 8
B_LOCAL = B_TOTAL // N_CORES  # 4
N_CT = C // 128  # 16 c-tiles per batch
N_G = 4  # groups of 4 c-tiles

_compiled = None


def _build():
    import concourse.bacc as bacc
    import concourse.tile as tile
    import concourse.mybir as mybir
    from concourse import masks

    f32 = mybir.dt.float32
    i32 = mybir.dt.int32

    nc = bacc.Bacc(
        "TRN2",
        target_bir_lowering=False,
        debug=False,
        num_devices=N_CORES,
    )

    ctx_d = nc.dram_tensor("context", [B_LOCAL, C, D], f32, kind="ExternalInput").ap()
    qry_d = nc.dram_tensor("query", [B_LOCAL, Q, D], f32, kind="ExternalInput").ap()
    w_d = nc.dram_tensor("w", [3 * D], f32, kind="ExternalInput").ap()
    msk_d = nc.dram_tensor("query_mask", [B_LOCAL, Q], i32, kind="ExternalInput").ap()
    out_d = nc.dram_tensor("out", [B_LOCAL, C, 4 * D], f32, kind="ExternalOutput").ap()

    with tile.TileContext(nc) as tc:
        _kernel_body(tc, out_d, ctx_d, qry_d, w_d, msk_d, mybir, masks)

    nc.compile()
    return nc


def _kernel_body(tc, out_d, ctx_d, qry_d, w_d, msk_d, mybir, masks):
    from contextlib import ExitStack

    nc = tc.nc
    f32 = mybir.dt.float32
    bf16 = mybir.dt.bfloat16
    f32r = mybir.dt.float32r
    i32 = mybir.dt.int32
    AFT = mybir.ActivationFunctionType
    Alu = mybir.AluOpType
    AX = mybir.AxisListType.X

    es = ExitStack()
    with es:
        # ---- pools ----
        consts = es.enter_context(tc.tile_pool(name="consts", bufs=1))
        outp = es.enter_context(tc.tile_pool(name="outp", bufs=4))
        bigs = es.enter_context(tc.tile_pool(name="bigs", bufs=2))
        meds = es.enter_context(tc.tile_pool(name="meds", bufs=2))
        cols = es.enter_context(tc.tile_pool(name="cols", bufs=8))
        ps_big = es.enter_context(tc.tile_pool(name="ps_big", bufs=2, space="PSUM"))
        ps_etr = es.enter_context(tc.tile_pool(name="ps_etr", bufs=2, space="PSUM"))
        ps_cq = es.enter_context(tc.tile_pool(name="ps_cq", bufs=3, space="PSUM"))
        ps_tail = es.enter_context(tc.tile_pool(name="ps_tail", bufs=1, space="PSUM"))

        # ---- constants ----
        # One SWDGE DMA for all of w as a contiguous row (1 descriptor; the
        # [d,1] column layout pays 7ns/desc min-transfer x 384), issued
        # before the identity builders so w is ready early.  Columns are
        # recovered with PE transposes below.
        w_row = consts.tile([1, 3 * D], f32)
        nc.gpsimd.dma_start(out=w_row[:], in_=w_d.rearrange("n -> () n"))
        ident = consts.tile([128, 128], f32)
        ident_bf = consts.tile([128, 128], bf16)
        ones_col = consts.tile([128, 1], f32)
        nc.vector.memset(ones_col[:], 1.0)
        ones_row_bf = consts.tile([1, 128], bf16)
        nc.vector.memset(ones_row_bf[:], 1.0)
        # all 4 batches' masks in one contiguous row: 1 descriptor instead of
        # 4x128 (the [q,1] column layout pays 7ns/desc min-transfer x 128)
        mask_rows = consts.tile([1, 4 * Q], i32)
        # p-major c mapping: c-tile i holds context rows c = p*16 + i, so
        # each partition covers 16 consecutive rows = 8 KiB contiguous DRAM.
        # That lets the bf16 casting ctx load use 4 KiB descriptors (full
        # DMA rate, halving ctx load bytes); compute is bf16-safe since the
        # exact f32 out1 block ships via DRAM->DRAM copy.
        ctx_v = ctx_d.rearrange("b (p i) d -> b p i d", i=N_CT)
        ctx_flat = ctx_d.rearrange("b (p i) d -> b p (i d)", i=N_CT)
        out_v = out_d.rearrange("b (p i) f -> b p i f", i=N_CT)

        # ---------- loads ----------
        # SP carries only the tiny qry/mask loads (stores dominate it later);
        # the bf16 ctx casting loads and the out1 d2d copies ride the SWDGE
        # queue, dependency-free, filling every DMA idle window.
        gts = []
        gvs = []
        qfs = []
        cbs = []
        for b in range(B_LOCAL):
            qf = meds.tile([128, 128], f32, tag="qf", bufs=4)
            gt = outp.tile([128, N_CT * 384], f32, tag="out")
            gv = gt.rearrange("p (i f) -> p i f", i=N_CT)
            cb = meds.tile([128, N_CT * 128], bf16, tag="ctxbf", bufs=4)
            gts.append(gt)
            gvs.append(gv)
            qfs.append(qf)
            cbs.append(cb)

        nc.sync.dma_start(out=qfs[0][:], in_=qry_d[0])
        # ctx_bf0 generates right after w_row on the SWDGE queue
        nc.gpsimd.dma_start(out=cbs[0][:], in_=ctx_flat[0])
        masks.make_identity(nc, ident[:])
        masks.make_identity(nc, ident_bf[:])
        nc.gpsimd.dma_start(out=out_v[0][:, :, 0:128], in_=ctx_v[0])
        nc.sync.dma_start(
            out=mask_rows[:], in_=msk_d.rearrange("b q -> () (b q)")
        )
        for b in range(1, B_LOCAL):
            nc.sync.dma_start(out=qfs[b][:], in_=qry_d[b])
        # recover w columns: PE transposes of the row slices + SBUF copies
        wps = ps_big.tile([128, 512], f32, tag="big")
        for k in range(3):
            nc.tensor.transpose(
                wps[:, k : k + 1], w_row[0:1, k * D : (k + 1) * D], ident[0:1, 0:1]
            )
        w1_col = consts.tile([128, 1], bf16)
        nc.vector.tensor_copy(w1_col[:], wps[:, 0:1])  # f32 -> bf16
        w2_col = consts.tile([128, 1], f32)
        nc.vector.tensor_copy(w2_col[:], wps[:, 1:2])
        w3_col = consts.tile([128, 1], f32)
        nc.vector.tensor_copy(w3_col[:], wps[:, 2:3])
        # SWDGE queue: remaining bf16 ctx casts + out1 d2d copies,
        # interleaved so the early ones are eligible as soon as possible
        nc.gpsimd.dma_start(out=cbs[1][:], in_=ctx_flat[1])
        nc.gpsimd.dma_start(out=out_v[1][:, :, 0:128], in_=ctx_v[1])
        nc.gpsimd.dma_start(out=cbs[2][:], in_=ctx_flat[2])
        nc.gpsimd.dma_start(out=out_v[2][:, :, 0:128], in_=ctx_v[2])
        nc.gpsimd.dma_start(out=cbs[3][:], in_=ctx_flat[3])
        nc.gpsimd.dma_start(out=out_v[3][:, :, 0:128], in_=ctx_v[3])

        for b in range(B_LOCAL):
            gv = gvs[b]
            cbv = cbs[b].rearrange("p (i d) -> p i d", i=N_CT)

            def ctx_blk(i):
                return cbv[:, i, :]

            def o_blk(i, k):
                return gv[:, i, (k - 1) * 128 : k * 128]

            # ---------- query prep ----------
            rhs_aug = meds.tile([128, 129], bf16, tag="rhs")
            nc.vector.tensor_copy(rhs_aug[:, 0:128], qfs[b][:])  # f32 -> bf16
            nc.vector.memset(rhs_aug[:, 128:129], 1.0)
            mrow_f = cols.tile([1, Q], f32, tag="mrowf")
            nc.vector.tensor_copy(
                mrow_f[:], mask_rows[0:1, b * Q : (b + 1) * Q]
            )  # int -> float cast

            # headA: qT (cols 0:128) + beta col (128) + mask col (129)
            headA = ps_big.tile([128, 512], f32, tag="big")
            nc.tensor.transpose(headA[:, 0:128], qfs[b][:], ident[:])
            nc.tensor.transpose(headA[:, 129:130], mrow_f[:], ident[0:1, 0:1])
            qT = meds.tile([128, 128], f32, tag="qT")
            nc.vector.tensor_copy(qT[:], headA[:, 0:128])
            qw3T = meds.tile([128, 128], bf16, tag="qw3T")
            nc.vector.tensor_scalar_mul(qw3T[:], qT[:], w3_col[:])
            madd_col = cols.tile([128, 1], f32, tag="madd")
            nc.vector.tensor_scalar(
                madd_col[:], headA[:, 129:130], 1.0, 1.0e9,
                op0=Alu.subtract, op1=Alu.mult,
            )
            nc.tensor.matmul(
                headA[:, 128:129], qT[:], w2_col[:], start=True, stop=True
            )
            beta_col = cols.tile([128, 1], f32, tag="beta")
            nc.vector.tensor_add(beta_col[:], madd_col[:], headA[:, 128:129])

            # per-batch tiles
            ctxT = bigs.tile([128, C], bf16, tag="ctxT")
            e_t = bigs.tile([128, C], bf16, tag="et")
            e_alpha = meds.tile([128, N_CT], f32, tag="ealpha")
            # bf16: pairs with the bf16 stationary ctx tile in the u matmuls
            e_m = meds.tile([128, N_CT], bf16, tag="em")
            # tail bank: u chain col 0, zb col 1, q2c row 2:130, bc 130:258
            tail_ps = ps_tail.tile([128, 512], f32, tag="tail")

            def stage_transposes(g):
                tr_ps = ps_big.tile([128, 512], bf16, tag="big")
                for j in range(4):
                    nc.tensor.transpose(
                        tr_ps[:, j * 128 : (j + 1) * 128],
                        ctx_blk(g * 4 + j),
                        ident_bf[:],
                    )
                # bf16 PSUM->SBUF copy hits DVE's 2x_1p fast path (392ns);
                # keeping all 4 on DVE takes them off the ACT group cadence
                nc.vector.tensor_copy(ctxT[:, g * 512 : (g + 1) * 512], tr_ps[:])

            # ---------- group-pipelined main loop ----------
            # Per group: G -> exp -> cq/etr -> scales/out3 -> store, with the
            # next group's ctx transposes staged one iteration ahead so the
            # PSUM->SBUF copy and exp latency never stall the PE.
            stage_transposes(0)
            for g in range(N_G):
                # G^T for this group (ctxT[g] copied last iteration)
                st_ps = ps_big.tile([128, 512], f32, tag="big")
                nc.tensor.matmul(
                    st_ps[:],
                    qw3T[:],
                    ctxT[:, g * 512 : (g + 1) * 512],
                    start=True,
                    stop=True,
                )
                if g + 1 < N_G:
                    stage_transposes(g + 1)
                nc.scalar.activation(
                    out=e_t[:, g * 512 : (g + 1) * 512],
                    in_=st_ps[:],
                    func=AFT.Exp,
                    bias=beta_col[:],
                    scale=1.0,
                )
                # cq PSUM allocs; alpha columns ride in cq2a cols 258:262
                cq2a = ps_cq.tile([128, 262], f32, tag="cq")
                for jj in range(4):
                    i = 4 * g + jj
                    nc.tensor.matmul(
                        cq2a[:, 258 + jj : 259 + jj],
                        ctxT[:, i * 128 : (i + 1) * 128],
                        w1_col[:],
                        start=True,
                        stop=True,
                    )
                # u chain for the previous group (e_m ready by then)
                if g > 0:
                    for i in range(4 * (g - 1), 4 * g):
                        nc.tensor.matmul(
                            tail_ps[:, 0:1],
                            ctx_blk(i),
                            e_m[:, i : i + 1],
                            start=(i == 0),
                            stop=False,
                        )
                etr = ps_etr.tile([128, 512], bf16, tag="etr")
                rzs = {}
                cq_slots = {}
                cq2 = cq2a
                for j in range(4):
                    i = 4 * g + j
                    if j == 2:
                        cq2 = ps_cq.tile([128, 262], f32, tag="cq")
                    cqs = cq2[:, 129 * (j % 2) : 129 * (j % 2) + 129]
                    cq_slots[j] = cqs
                    et_sl = e_t[:, i * 128 : (i + 1) * 128]
                    nc.tensor.matmul(cqs, et_sl, rhs_aug[:], start=True, stop=True)
                    nc.tensor.transpose(
                        etr[:, j * 128 : (j + 1) * 128], et_sl, ident_bf[:]
                    )
                    if j % 2 == 1:
                        # one reciprocal per pair: Z cols sit at 128 and 257
                        rz2 = cols.tile([128, 2], f32, tag="rz")
                        zv = cq2[:, 0:258].rearrange("p (k n) -> p k n", k=2)[
                            :, :, 128
                        ]
                        nc.vector.reciprocal(rz2[:], zv)
                        rzs[j - 1] = rz2[:, 0:1]
                        rzs[j] = rz2[:, 1:2]
                for j in range(4):
                    i = 4 * g + j
                    # c2q = (E @ [qry|1]) / Z  (ACT copy with per-partition scale)
                    nc.scalar.activation(
                        out=o_blk(i, 1),
                        in_=cq_slots[j][:, 0:128],
                        func=AFT.Copy,
                        scale=rzs[j],
                    )
                # e^alpha for this group's 4 tiles (not on the store path)
                nc.scalar.activation(
                    out=e_alpha[:, 4 * g : 4 * g + 4],
                    in_=cq2a[:, 258:262],
                    func=AFT.Exp,
                )
                # row max over q of the 4 transposed tiles, then e_m
                maxE = cols.tile([128, 4], f32, tag="maxE")
                nc.vector.reduce_max(
                    out=maxE[:], in_=etr.rearrange("p (j q) -> p j q", j=4), axis=AX
                )
                nc.vector.tensor_mul(
                    e_m[:, 4 * g : 4 * g + 4], e_alpha[:, 4 * g : 4 * g + 4], maxE[:]
                )
                # out3 = ctx * c2q (SBUF-only: gpsimd can help)
                for j in range(4):
                    i = 4 * g + j
                    eng = nc.gpsimd if j >= 2 else nc.vector
                    eng.tensor_mul(o_blk(i, 2), ctx_blk(i), o_blk(i, 1))
                # this group's [c2q | ctx*c2q] is final: ship it
                # (batch 0 group 0 in two halves so DMA starts sooner)
                if b == 0 and g == 0:
                    nc.sync.dma_start(
                        out=out_v[b][:, 0:2, 128:384], in_=gv[:, 0:2, 0:256]
                    )
                    nc.sync.dma_start(
                        out=out_v[b][:, 2:4, 128:384], in_=gv[:, 2:4, 0:256]
                    )
                else:
                    nc.sync.dma_start(
                        out=out_v[b][:, 4 * g : 4 * g + 4, 128:384],
                        in_=gv[:, 4 * g : 4 * g + 4, 0:256],
                    )
            for i in range(4 * (N_G - 1), N_CT):
                nc.tensor.matmul(
                    tail_ps[:, 0:1],
                    ctx_blk(i),
                    e_m[:, i : i + 1],
                    start=False,
                    stop=(i == N_CT - 1),
                )

            # ---------- q2c epilogue ----------
            zsum = cols.tile([128, 1], f32, tag="zsum")
            nc.vector.reduce_sum(out=zsum[:], in_=e_m[:], axis=AX)
            nc.tensor.matmul(
                tail_ps[0:1, 1:2], zsum[:], ones_col[:], start=True, stop=True
            )
            u_sb = cols.tile([128, 1], f32, tag="usb")
            nc.vector.tensor_copy(u_sb[:], tail_ps[:, 0:1])
            nc.tensor.transpose(tail_ps[0:1, 2:130], u_sb[:], ident[:])
            rzb = cols.tile([1, 1], f32, tag="rzb")
            nc.vector.reciprocal(rzb[:], tail_ps[0:1, 1:2])
            q2c_row = cols.tile([1, 128], bf16, tag="q2crow")
            nc.scalar.activation(
                out=q2c_row[:], in_=tail_ps[0:1, 2:130], func=AFT.Copy, scale=rzb[:]
            )
            nc.tensor.matmul(
                tail_ps[:, 130:258], ones_row_bf[:], q2c_row[:], start=True, stop=True
            )
            q2c_sb = meds.tile([128, 128], f32, tag="q2csb")
            nc.vector.tensor_copy(q2c_sb[:], tail_ps[:, 130:258])

            # ---------- out4 + remaining stores (4 pieces) ----------
            # 3 DVE + 1 gpsimd mul per piece; each piece ships on completion
            for piece in range(4):
                for j in range(4):
                    i = 4 * piece + j
                    eng = nc.gpsimd if j == 1 else nc.vector
                    eng.tensor_mul(o_blk(i, 3), ctx_blk(i), q2c_sb[:])
                nc.sync.dma_start(
                    out=out_v[b][:, 4 * piece : 4 * piece + 4, 384:512],
                    in_=gv[:, 4 * piece : 4 * piece + 4, 256:384],
                )


def kernel(**inputs):
    global _compiled
    from concourse.bass_utils import run_bass_kernel_spmd

    context = np.ascontiguousarray(inputs["context"], dtype=np.float32)
    query = np.ascontiguousarray(inputs["query"], dtype=np.float32)
    w = np.ascontiguousarray(inputs["w"], dtype=np.float32)
    qmask = np.ascontiguousarray(inputs["query_mask"], dtype=np.int32)

    if _compiled is None:
        _compiled = _build()
    nc = _compiled

    core_ids = list(range(N_CORES))
    in_maps = []
    for k in core_ids:
        sl = slice(k * B_LOCAL, (k + 1) * B_LOCAL)
        in_maps.append(
            {
                "context": context[sl],
                "query": query[sl],
                "w": w,
                "query_mask": qmask[sl],
            }
        )

    res = run_bass_kernel_spmd(nc, in_maps, core_ids)
    outs = [res.results[k]["out"] for k in range(N_CORES)]
    return np.concatenate(outs, axis=0)


# revision 60
# speedup vs baseline: 1.0696x; 1.0211x over previous
# ContextQueryAttention (BiDAF-style) Trainium2 Bass/Tile kernel.
#
# Full-input contract: kernel(**inputs) takes the full arrays
#   context [32, 2048, 128] f32, query [32, 128, 128] f32,
#   w [384] f32, query_mask [32, 128] i32
# and returns out [32, 2048, 512] f32.
#
# Sharding: batch B=32 split 4-per-core across 8 NeuronCores (pure data
# parallel, no collectives).
#
# Math (per batch, C=2048, Q=128, D=128):
#   S[c,q] = ctx[c]@w1 + query[q]@w2 + (ctx[c]*w3)@query[q]
#          = alpha[c] + beta[q] + G[c,q]
#   a = softmax_q(S + maskadd);  c2q = a @ query
#   m[c] = max_q(S + maskadd);   b = softmax_c(m); q2c = b @ ctx
#   out = [ctx | c2q | ctx*c2q | ctx*q2c]
#
# Design notes (cost-model driven; DMA floor = 20.2 MiB/core at
# 360 GB/s = 59.0 us, the DMA engine runs gap-free start to end and the
# total is floor + fixed startup/drain only):
#  * alpha[c] cancels in softmax_q -> row softmax runs on T = G + beta'
#    (beta' = beta + mask_add) fused into the ACT exp bias in [q, c] layout.
#  * |S| = O(5), so exp() without max-subtraction is exact to fp32 roundoff.
#  * bf16 everywhere precision allows (rel-err budget 2e-2, this costs
#    ~5e-4): ctxT/qw3T for the G matmul, E^T = exp(T^T), the c2q matmuls
#    and E transposes -> 1 cyc/row on the PE instead of 4 (fp32).
#  * max_q E per c-tile via PE-transpose of E^T; 4 tiles transposed into
#    one PSUM bank and reduced with a single 3D reduce_max.
#  * u = sum_c e_m[c]*ctx[c] computed transposed: stationary ctx tile,
#    moving e_m column -> N=1 matmuls (~free on PE).
#  * out[:, :, 0:128] == ctx exactly, so it ships as dependency-free
#    DRAM->DRAM copies on the gpsimd SWDGE queue; they fill every DMA
#    idle window (the cost model charges only output bytes).
#  * Group-pipelined batch body: per group of 4 c-tiles, G -> exp -> cq/
#    etr -> scales/out3 -> store, with the next group's ctx transposes
#    staged one iteration ahead.  First store of a batch comes ~2.5 us
#    after the batch starts, so the DMA queue never starves at batch
#    boundaries (DMA engine ends up >94% busy, gap-free after startup).
#  * One SBUF assembly tile per batch [128, 16*512]; ctx loads land in its
#    first column block; stores are per-group (cols 128:384) plus 4
#    out4 pieces (cols 384:512).  All loads are issued before any store
#    on the SP queue so a waiting store never blocks a later load; w and
#    the out1 d2d copies ride the SWDGE queue (no HWDGE contention).
#  * Elementwise work is spread: exp/scales on ACT, muls/reduces/recips
#    on DVE, part of out3/out4 on gpsimd (Pool).  Paired reciprocals
#    (2 Z columns per DVE op) halve the per-tile recip overhead.
#
# PSUM (8 banks): big 2 (head / ctx transposes / G) + etr 2 (E-transpose
# groups) + cq 3 (c2q results 2-per-bank + alpha columns) + tail 1
# (u accumulation chain, zb, q2c row, broadcast).

import numpy as np

C = 2048
Q = 128
D = 128
B_TOTAL = 32
N_CORES = 8
B_LOCAL = B_TOTAL // N_CORES  # 4
N_CT = C // 128  # 16 c-tiles per batch
N_G = 4  # groups of 4 c-tiles

_compiled = None


def _build():
    import concourse.bacc as bacc
    import concourse.tile as tile
    import concourse.mybir as mybir
    from concourse import masks

    f32 = mybir.dt.float32
    i32 = mybir.dt.int32

    nc = bacc.Bacc(
        "TRN2",
        target_bir_lowering=False,
        debug=False,
        num_devices=N_CORES,
    )

    ctx_d = nc.dram_tensor("context", [B_LOCAL, C, D], f32, kind="ExternalInput").ap()
    qry_d = nc.dram_tensor("query", [B_LOCAL, Q, D], f32, kind="ExternalInput").ap()
    w_d = nc.dram_tensor("w", [3 * D], f32, kind="ExternalInput").ap()
    msk_d = nc.dram_tensor("query_mask", [B_LOCAL, Q], i32, kind="ExternalInput").ap()
    out_d = nc.dram_tensor("out", [B_LOCAL, C, 4 * D], f32, kind="ExternalOutput").ap()

    with tile.TileContext(nc) as tc:
        _kernel_body(tc, out_d, ctx_d, qry_d, w_d, msk_d, mybir, masks)

    nc.compile()
    return nc


def _kernel_body(tc, out_d, ctx_d, qry_d, w_d, msk_d, mybir, masks):
    from contextlib import ExitStack

    nc = tc.nc
    f32 = mybir.dt.float32
    bf16 = mybir.dt.bfloat16
    f32r = mybir.dt.float32r
    i32 = mybir.dt.int32
    AFT = mybir.ActivationFunctionType
    Alu = mybir.AluOpType
    AX = mybir.AxisListType.X

    es = ExitStack()
    with es:
        # ---- pools ----
        consts = es.enter_context(tc.tile_pool(name="consts", bufs=1))
        outp = es.enter_context(tc.tile_pool(name="outp", bufs=4))
        bigs = es.enter_context(tc.tile_pool(name="bigs", bufs=2))
        meds = es.enter_context(tc.tile_pool(name="meds", bufs=2))
        cols = es.enter_context(tc.tile_pool(name="cols", bufs=8))
        ps_big = es.enter_context(tc.tile_pool(name="ps_big", bufs=2, space="PSUM"))
        ps_etr = es.enter_context(tc.tile_pool(name="ps_etr", bufs=2, space="PSUM"))
        ps_cq = es.enter_context(tc.tile_pool(name="ps_cq", bufs=3, space="PSUM"))
        ps_tail = es.enter_context(tc.tile_pool(name="ps_tail", bufs=1, space="PSUM"))

        # ---- constants ----
        # One SWDGE DMA for all of w as a contiguous row (1 descriptor; the
        # [d,1] column layout pays 7ns/desc min-transfer x 384), issued
        # before the identity builders so w is ready early.  Columns are
        # recovered with PE transposes below.
        w_row = consts.tile([1, 3 * D], f32)
        nc.gpsimd.dma_start(out=w_row[:], in_=w_d.rearrange("n -> () n"))
        ident = consts.tile([128, 128], f32)
        ident_bf = consts.tile([128, 128], bf16)
        ones_col = consts.tile([128, 1], f32)
        nc.vector.memset(ones_col[:], 1.0)
        ones_row_bf = consts.tile([1, 128], bf16)
        nc.vector.memset(ones_row_bf[:], 1.0)
        # all 4 batches' masks in one contiguous row: 1 descriptor instead of
        # 4x128 (the [q,1] column layout pays 7ns/desc min-transfer x 128)
        mask_rows = consts.tile([1, 4 * Q], i32)
        # p-major c mapping: c-tile i holds context rows c = p*16 + i, so
        # each partition covers 16 consecutive rows = 8 KiB contiguous DRAM.
        # That lets the bf16 casting ctx load use 4 KiB descriptors (full
        # DMA rate, halving ctx load bytes); compute is bf16-safe since the
        # exact f32 out1 block ships via DRAM->DRAM copy.
        ctx_v = ctx_d.rearrange("b (p i) d -> b p i d", i=N_CT)
        ctx_flat = ctx_d.rearrange("b (p i) d -> b p (i d)", i=N_CT)
        out_v = out_d.rearrange("b (p i) f -> b p i f", i=N_CT)

        # ---------- loads ----------
        # SP carries only the tiny qry/mask loads (stores dominate it later);
        # the bf16 ctx casting loads and the out1 d2d copies ride the SWDGE
        # queue, dependency-free, filling every DMA idle window.
        gts = []
        gvs = []
        qfs = []
        cbs = []
        for b in range(B_LOCAL):
            qf = meds.tile([128, 128], f32, tag="qf", bufs=4)
            gt = outp.tile([128, N_CT * 384], f32, tag="out")
            gv = gt.rearrange("p (i f) -> p i f", i=N_CT)
            cb = meds.tile([128, N_CT * 128], bf16, tag="ctxbf", bufs=4)
            gts.append(gt)
            gvs.append(gv)
            qfs.append(qf)
            cbs.append(cb)

        nc.sync.dma_start(out=qfs[0][:], in_=qry_d[0])
        # ctx_bf0 generates right after w_row on the SWDGE queue
        nc.gpsimd.dma_start(out=cbs[0][:], in_=ctx_flat[0])
        masks.make_identity(nc, ident[:])
        masks.make_identity(nc, ident_bf[:])
        nc.gpsimd.dma_start(out=out_v[0][:, :, 0:128], in_=ctx_v[0])
        nc.sync.dma_start(
            out=mask_rows[:], in_=msk_d.rearrange("b q -> () (b q)")
        )
        for b in range(1, B_LOCAL):
            nc.sync.dma_start(out=qfs[b][:], in_=qry_d[b])
        # recover w columns: PE transposes of the row slices + SBUF copies
        wps = ps_big.tile([128, 512], f32, tag="big")
        for k in range(3):
            nc.tensor.transpose(
                wps[:, k : k + 1], w_row[0:1, k * D : (k + 1) * D], ident[0:1, 0:1]
            )
        w1_col = consts.tile([128, 1], bf16)
        nc.vector.tensor_copy(w1_col[:], wps[:, 0:1])  # f32 -> bf16
        w2_col = consts.tile([128, 1], f32)
        nc.vector.tensor_copy(w2_col[:], wps[:, 1:2])
        w3_col = consts.tile([128, 1], f32)
        nc.vector.tensor_copy(w3_col[:], wps[:, 2:3])
        # SWDGE queue: remaining bf16 ctx casts + out1 d2d copies,
        # interleaved so the early ones are eligible as soon as possible
        nc.gpsimd.dma_start(out=cbs[1][:], in_=ctx_flat[1])
        nc.gpsimd.dma_start(out=out_v[1][:, :, 0:128], in_=ctx_v[1])
        nc.gpsimd.dma_start(out=cbs[2][:], in_=ctx_flat[2])
        nc.gpsimd.dma_start(out=out_v[2][:, :, 0:128], in_=ctx_v[2])
        nc.gpsimd.dma_start(out=cbs[3][:], in_=ctx_flat[3])
        nc.gpsimd.dma_start(out=out_v[3][:, :, 0:128], in_=ctx_v[3])

        for b in range(B_LOCAL):
            gv = gvs[b]
            cbv = cbs[b].rearrange("p (i d) -> p i d", i=N_CT)

            def ctx_blk(i):
                return cbv[:, i, :]

            def o_blk(i, k):
                return gv[:, i, (k - 1) * 128 : k * 128]

            # ---------- query prep ----------
            rhs_aug = meds.tile([128, 129], bf16, tag="rhs")
            nc.vector.tensor_copy(rhs_aug[:, 0:128], qfs[b][:])  # f32 -> bf16
            nc.vector.memset(rhs_aug[:, 128:129], 1.0)
            mrow_f = cols.tile([1, Q], f32, tag="mrowf")
            nc.vector.tensor_copy(
                mrow_f[:], mask_rows[0:1, b * Q : (b + 1) * Q]
            )  # int -> float cast

            # headA: qT (cols 0:128) + beta col (128) + mask col (129)
            headA = ps_big.tile([128, 512], f32, tag="big")
            nc.tensor.transpose(headA[:, 0:128], qfs[b][:], ident[:])
            nc.tensor.transpose(headA[:, 129:130], mrow_f[:], ident[0:1, 0:1])
            qT = meds.tile([128, 128], f32, tag="qT")
            nc.vector.tensor_copy(qT[:], headA[:, 0:128])
            qw3T = meds.tile([128, 128], bf16, tag="qw3T")
            nc.vector.tensor_scalar_mul(qw3T[:], qT[:], w3_col[:])
            madd_col = cols.tile([128, 1], f32, tag="madd")
            nc.vector.tensor_scalar(
                madd_col[:], headA[:, 129:130], 1.0, 1.0e9,
                op0=Alu.subtract, op1=Alu.mult,
            )
            nc.tensor.matmul(
                headA[:, 128:129], qT[:], w2_col[:], start=True, stop=True
            )
            beta_col = cols.tile([128, 1], f32, tag="beta")
            nc.vector.tensor_add(beta_col[:], madd_col[:], headA[:, 128:129])

            # per-batch tiles
            ctxT = bigs.tile([128, C], bf16, tag="ctxT")
            e_t = bigs.tile([128, C], bf16, tag="et")
            e_alpha = meds.tile([128, N_CT], f32, tag="ealpha")
            # bf16: pairs with the bf16 stationary ctx tile in the u matmuls
            e_m = meds.tile([128, N_CT], bf16, tag="em")
            # tail bank: u chain col 0, zb col 1, q2c row 2:130, bc 130:258
            tail_ps = ps_tail.tile([128, 512], f32, tag="tail")

            def stage_transposes(g):
                tr_ps = ps_big.tile([128, 512], bf16, tag="big")
                for j in range(4):
                    nc.tensor.transpose(
                        tr_ps[:, j * 128 : (j + 1) * 128],
                        ctx_blk(g * 4 + j),
                        ident_bf[:],
                    )
                # bf16 PSUM->SBUF copy: DVE side hits the 2x_1p fast path
                if g % 2 == 0:
                    nc.scalar.copy(ctxT[:, g * 512 : (g + 1) * 512], tr_ps[:])
                else:
                    nc.vector.tensor_copy(ctxT[:, g * 512 : (g + 1) * 512], tr_ps[:])

            # ---------- group-pipelined main loop ----------
            # Per group: G -> exp -> cq/etr -> scales/out3 -> store, with the
            # next group's ctx transposes staged one iteration ahead so the
            # PSUM->SBUF copy and exp latency never stall the PE.
            stage_transposes(0)
            for g in range(N_G):
                # G^T for this group (ctxT[g] copied last iteration)
                st_ps = ps_big.tile([128, 512], f32, tag="big")
                nc.tensor.matmul(
                    st_ps[:],
                    qw3T[:],
                    ctxT[:, g * 512 : (g + 1) * 512],
                    start=True,
                    stop=True,
                )
                if g + 1 < N_G:
                    stage_transposes(g + 1)
                nc.scalar.activation(
                    out=e_t[:, g * 512 : (g + 1) * 512],
                    in_=st_ps[:],
                    func=AFT.Exp,
                    bias=beta_col[:],
                    scale=1.0,
                )
                # cq PSUM allocs; alpha columns ride in cq2a cols 258:262
                cq2a = ps_cq.tile([128, 262], f32, tag="cq")
                for jj in range(4):
                    i = 4 * g + jj
                    nc.tensor.matmul(
                        cq2a[:, 258 + jj : 259 + jj],
                        ctxT[:, i * 128 : (i + 1) * 128],
                        w1_col[:],
                        start=True,
                        stop=True,
                    )
                # u chain for the previous group (e_m ready by then)
                if g > 0:
                    for i in range(4 * (g - 1), 4 * g):
                        nc.tensor.matmul(
                            tail_ps[:, 0:1],
                            ctx_blk(i),
                            e_m[:, i : i + 1],
                            start=(i == 0),
                            stop=False,
                        )
                etr = ps_etr.tile([128, 512], bf16, tag="etr")
                rzs = {}
                cq_slots = {}
                cq2 = cq2a
                for j in range(4):
                    i = 4 * g + j
                    if j == 2:
                        cq2 = ps_cq.tile([128, 262], f32, tag="cq")
                    cqs = cq2[:, 129 * (j % 2) : 129 * (j % 2) + 129]
                    cq_slots[j] = cqs
                    et_sl = e_t[:, i * 128 : (i + 1) * 128]
                    nc.tensor.matmul(cqs, et_sl, rhs_aug[:], start=True, stop=True)
                    nc.tensor.transpose(
                        etr[:, j * 128 : (j + 1) * 128], et_sl, ident_bf[:]
                    )
                    if j % 2 == 1:
                        # one reciprocal per pair: Z cols sit at 128 and 257
                        rz2 = cols.tile([128, 2], f32, tag="rz")
                        zv = cq2[:, 0:258].rearrange("p (k n) -> p k n", k=2)[
                            :, :, 128
                        ]
                        nc.vector.reciprocal(rz2[:], zv)
                        rzs[j - 1] = rz2[:, 0:1]
                        rzs[j] = rz2[:, 1:2]
                for j in range(4):
                    i = 4 * g + j
                    # c2q = (E @ [qry|1]) / Z  (ACT copy with per-partition scale)
                    nc.scalar.activation(
                        out=o_blk(i, 1),
                        in_=cq_slots[j][:, 0:128],
                        func=AFT.Copy,
                        scale=rzs[j],
                    )
                # e^alpha for this group's 4 tiles (not on the store path)
                nc.scalar.activation(
                    out=e_alpha[:, 4 * g : 4 * g + 4],
                    in_=cq2a[:, 258:262],
                    func=AFT.Exp,
                )
                # row max over q of the 4 transposed tiles, then e_m
                maxE = cols.tile([128, 4], f32, tag="maxE")
                nc.vector.reduce_max(
                    out=maxE[:], in_=etr.rearrange("p (j q) -> p j q", j=4), axis=AX
                )
                nc.vector.tensor_mul(
                    e_m[:, 4 * g : 4 * g + 4], e_alpha[:, 4 * g : 4 * g + 4], maxE[:]
                )
                # out3 = ctx * c2q (SBUF-only: gpsimd can help)
                for j in range(4):
                    i = 4 * g + j
                    eng = nc.gpsimd if j >= 2 else nc.vector
                    eng.tensor_mul(o_blk(i, 2), ctx_blk(i), o_blk(i, 1))
                # this group's [c2q | ctx*c2q] is final: ship it
                # (batch 0 group 0 in two halves so DMA starts sooner)
                if b == 0 and g == 0:
                    nc.sync.dma_start(
                        out=out_v[b][:, 0:2, 128:384], in_=gv[:, 0:2, 0:256]
                    )
                    nc.sync.dma_start(
                        out=out_v[b][:, 2:4, 128:384], in_=gv[:, 2:4, 0:256]
                    )
                else:
                    nc.sync.dma_start(
                        out=out_v[b][:, 4 * g : 4 * g + 4, 128:384],
                        in_=gv[:, 4 * g : 4 * g + 4, 0:256],
                    )
            for i in range(4 * (N_G - 1), N_CT):
                nc.tensor.matmul(
                    tail_ps[:, 0:1],
                    ctx_blk(i),
                    e_m[:, i : i + 1],
                    start=False,
                    stop=(i == N_CT - 1),
                )

            # ---------- q2c epilogue ----------
            zsum = cols.tile([128, 1], f32, tag="zsum")
            nc.vector.reduce_sum(out=zsum[:], in_=e_m[:], axis=AX)
            nc.tensor.matmul(
                tail_ps[0:1, 1:2], zsum[:], ones_col[:], start=True, stop=True
            )
            u_sb = cols.tile([128, 1], f32, tag="usb")
            nc.vector.tensor_copy(u_sb[:], tail_ps[:, 0:1])
            nc.tensor.transpose(tail_ps[0:1, 2:130], u_sb[:], ident[:])
            rzb = cols.tile([1, 1], f32, tag="rzb")
            nc.vector.reciprocal(rzb[:], tail_ps[0:1, 1:2])
            q2c_row = cols.tile([1, 128], bf16, tag="q2crow")
            nc.scalar.activation(
                out=q2c_row[:], in_=tail_ps[0:1, 2:130], func=AFT.Copy, scale=rzb[:]
            )
            nc.tensor.matmul(
                tail_ps[:, 130:258], ones_row_bf[:], q2c_row[:], start=True, stop=True
            )
            q2c_sb = meds.tile([128, 128], f32, tag="q2csb")
            nc.vector.tensor_copy(q2c_sb[:], tail_ps[:, 130:258])

            # ---------- out4 + remaining stores (4 pieces) ----------
            # 3 DVE + 1 gpsimd mul per piece; each piece ships on completion
            for piece in range(4):
                for j in range(4):
                    i = 4 * piece + j
                    eng = nc.gpsimd if j == 1 else nc.vector
                    eng.tensor_mul(o_blk(i, 3), ctx_blk(i), q2c_sb[:])
                nc.sync.dma_start(
                    out=out_v[b][:, 4 * piece : 4 * piece + 4, 384:512],
                    in_=gv[:, 4 * piece : 4 * piece + 4, 256:384],
                )


def kernel(**inputs):
    global _compiled
    from concourse.bass_utils import run_bass_kernel_spmd

    context = np.ascontiguousarray(inputs["context"], dtype=np.float32)
    query = np.ascontiguousarray(inputs["query"], dtype=np.float32)
    w = np.ascontiguousarray(inputs["w"], dtype=np.float32)
    qmask = np.ascontiguousarray(inputs["query_mask"], dtype=np.int32)

    if _compiled is None:
        _compiled = _build()
    nc = _compiled

    core_ids = list(range(N_CORES))
    in_maps = []
    for k in core_ids:
        sl = slice(k * B_LOCAL, (k + 1) * B_LOCAL)
        in_maps.append(
            {
                "context": context[sl],
                "query": query[sl],
                "w": w,
                "query_mask": qmask[sl],
            }
        )

    res = run_bass_kernel_spmd(nc, in_maps, core_ids)
    outs = [res.results[k]["out"] for k in range(N_CORES)]
    return np.concatenate(outs, axis=0)
